# revision 15
# baseline (speedup 1.0000x reference)
"""Trainium2 Bass kernel for the DistillationLoss problem.

Strategy (data parallel over batch, 8 cores x 4 samples):
  total = ALPHA*distill + (1-ALPHA)*(task_seg + task_pose)

  * seg_distill is identically 0 (softmax over a single channel), so
    t_seg_logits is never read.
  * pose_distill per sample reduces to scalars computed in one streaming
    pass over s_pose/t_pose:
        Zs = sum exp(s/T), Zt = sum exp(t/T),
        A1 = sum exp(t/T)*t, A2 = sum exp(t/T)*s
        KL_b = (A1-A2)/(T*Zt) - ln Zt + ln Zs
    (logits ~ N(0,1) so exp without max-subtraction is safe in fp32)
  * keypoints MSE per sample decomposes as S2 - 2*M2 + T2 with
        S2 = sum s^2                      (device, streaming pass)
        M2 = sum_{p} gx_p^T S gy_p        (device, PE matmuls vs transposed
                                           gaussian factors; avoids ever
                                           materializing the target heatmaps)
        T2 = sum tg^2                     (host, tiny: function of keypoints only)
  * BCE uses softplus(x) - x*m; softplus via Ln(exp(x)+1) on ACT.

Device returns per-partition partial sums; the host reduces in float64.
"""

import numpy as np
from contextlib import ExitStack

import concourse.bass as bass
import concourse.bacc as bacc
import concourse.tile as tile
from concourse import mybir
from concourse.bass_utils import run_bass_kernel_spmd

F32 = mybir.dt.float32
AF = mybir.ActivationFunctionType
ALU = mybir.AluOpType

B, P, K, H, W = 32, 8, 17, 192, 192
ALPHA, TEMP, SIGMA = 0.5, 2.0, 3.0
INV2S2 = 1.0 / (2.0 * SIGMA * SIGMA)
NCORES = 8
BPC = B // NCORES            # samples per core (4)
NPAIR = BPC // 2             # sample pairs per core (2)
KP = BPC * K * P             # gaussian rows per core (544)
KP_TILES = (KP + 127) // 128  # 5
KCH = [(0, 6), (6, 6), (12, 5)]  # k-chunks of the K=17 axis
SEG_F = BPC * H * W // 128   # free dim of flattened seg tiles (1152)

# ---- stat column maps (shared by device builder and host reducer) ----
# stats_act[128, 64]: per (pair, chunk, grp) slot: ZS, ZT ; then SP
# stats_dve[128, 64]: per slot: A1, A2 ; then 8x M2 ; then XM
# stats_gp [128, 32]: per slot: S2
NSLOT = NPAIR * 3 * 3


def _slot(pair, chunk, grp):
    return (pair * 3 + chunk) * 3 + grp


def _c_zs(s): return 2 * s
def _c_zt(s): return 2 * s + 1
C_SP = 2 * NSLOT            # 36
def _c_a1(s): return 2 * s
def _c_a2(s): return 2 * s + 1
def _c_m2(pair, bi, jc): return 2 * NSLOT + (pair * 2 + bi) * 2 + jc  # 36..43
C_XM = 2 * NSLOT + NPAIR * 2 * 2            # 36 + 8 = 44
def _c_s2(s): return s

PARTIALS_COLS = 160  # [0:64] act, [64:128] dve, [128:160] gp


def build_nc(s2_engine="act", en_stream=True, en_m2=True, en_bce=True):
    nc = bacc.Bacc("TRN2", target_bir_lowering=False)

    sp = nc.dram_tensor("s_pose", [BPC, K, H, W], F32, kind="ExternalInput")
    tp = nc.dram_tensor("t_pose", [BPC, K, H, W], F32, kind="ExternalInput")
    sg = nc.dram_tensor("s_seg", [BPC, H, W], F32, kind="ExternalInput")
    mk = nc.dram_tensor("mask", [BPC, H, W], F32, kind="ExternalInput")
    bxd = nc.dram_tensor("bx", [128, KP_TILES], F32, kind="ExternalInput")
    byd = nc.dram_tensor("by", [128, KP_TILES], F32, kind="ExternalInput")
    out_d = nc.dram_tensor("partials", [128, PARTIALS_COLS], F32, kind="ExternalOutput")

    iota_c = nc.inline_tensor(
        np.tile(np.arange(W, dtype=np.float32), (128, 1)), name="iota_c")
    ident_c = nc.inline_tensor(np.eye(128, dtype=np.float32), name="ident_c")

    with tile.TileContext(nc) as tc, ExitStack() as ctx:
        const = ctx.enter_context(tc.tile_pool(name="const", bufs=1))
        gnat = ctx.enter_context(tc.tile_pool(name="gnat", bufs=4))
        tps = ctx.enter_context(tc.tile_pool(name="tps", bufs=2, space="PSUM"))
        spool = ctx.enter_context(tc.tile_pool(name="spool", bufs=6))
        tpool = ctx.enter_context(tc.tile_pool(name="tpool", bufs=6))
        epool = ctx.enter_context(tc.tile_pool(name="epool", bufs=3))
        jpool = ctx.enter_context(tc.tile_pool(name="jpool", bufs=4))
        m2ps = ctx.enter_context(tc.tile_pool(name="m2ps", bufs=2, space="PSUM"))
        mjp = ctx.enter_context(tc.tile_pool(name="mjp", bufs=2))
        bpool = ctx.enter_context(tc.tile_pool(name="bpool", bufs=2))

        # ---- constants ----
        iota_t = const.tile([128, W], F32)
        nc.sync.dma_start(out=iota_t, in_=iota_c[:, :])
        ident_t = const.tile([128, 128], F32)
        nc.sync.dma_start(out=ident_t, in_=ident_c[:, :])
        bx_t = const.tile([128, KP_TILES], F32)
        nc.sync.dma_start(out=bx_t, in_=bxd[:, :])
        by_t = const.tile([128, KP_TILES], F32)
        nc.sync.dma_start(out=by_t, in_=byd[:, :])

        # ---- stats tiles ----
        stats_act = const.tile([128, 64], F32)
        stats_dve = const.tile([128, 64], F32)
        stats_gp = const.tile([128, 32], F32)
        nc.gpsimd.memset(stats_act, 0.0)
        nc.gpsimd.memset(stats_dve, 0.0)
        nc.gpsimd.memset(stats_gp, 0.0)

        # ---- gaussian factors, transposed: g[row, col] -> gT[coord, row] ----
        # gxT1 [128,KP]: coords h 0:128 ; gxT2r [128,KP]: h 128:192 in parts
        # 0:64 AND replicated into parts 64:128 (so both halves of packed B01
        # tiles find a partition-aligned rhs).
        gxT1 = const.tile([128, KP], F32)
        gxT2r = const.tile([128, KP], F32)
        gyT1 = const.tile([128, KP], F32)
        gyT2 = const.tile([64, KP], F32)

        for (bias_t, gT1, gT2, repl) in ((bx_t, gxT1, gxT2r, True),
                                         (by_t, gyT1, gyT2, False)):
            for t in range(KP_TILES):
                sz = min(128, KP - t * 128)
                off = t * 128
                gsq = gnat.tile([128, W], F32, tag="gsq")
                nc.scalar.activation(out=gsq[:sz], in_=iota_t[:sz],
                                     func=AF.Square, bias=bias_t[:sz, t:t + 1],
                                     scale=1.0)
                gex = gnat.tile([128, W], F32, tag="gex")
                nc.scalar.activation(out=gex[:sz], in_=gsq[:sz],
                                     func=AF.Exp, scale=-INV2S2)
                pt1 = tps.tile([128, 128], F32, tag="pt1")
                nc.tensor.transpose(out=pt1[:128, :sz], in_=gex[:sz, 0:128],
                                    identity=ident_t[:sz, :sz])
                nc.vector.tensor_copy(out=gT1[:, off:off + sz], in_=pt1[:128, :sz])
                pt2 = tps.tile([128, 128], F32, tag="pt2")
                nc.tensor.transpose(out=pt2[:64, :sz], in_=gex[:sz, 128:192],
                                    identity=ident_t[:sz, :sz])
                nc.vector.tensor_copy(out=gT2[0:64, off:off + sz], in_=pt2[:64, :sz])
            if repl:
                nc.sync.dma_start(out=gT2[64:128, 0:KP], in_=gT2[0:64, 0:KP])

        # ---- streaming pass + M2 matmuls ----
        for pair in range(NPAIR):
            bb = (2 * pair, 2 * pair + 1)
            ps = {}
            for bi in range(2):
                ps[(bi, 0)] = m2ps.tile([128, K * P], F32, tag="psj0",
                                        name=f"ps{pair}_{bi}_0")
                ps[(bi, 1)] = m2ps.tile([64, K * P], F32, tag="psj1",
                                        name=f"ps{pair}_{bi}_1")

            for ci, (k0, kn) in enumerate(KCH):
                for gi in range(3):  # 0: b0 h<128, 1: b1 h<128, 2: packed h>=128
                    s_t = spool.tile([128, 6, W], F32, tag="s")
                    t_t = tpool.tile([128, 6, W], F32, tag="t")
                    if gi < 2:
                        b = bb[gi]
                        nc.sync.dma_start(
                            out=s_t[:, :kn, :],
                            in_=sp[b, k0:k0 + kn, 0:128, :].rearrange("k h w -> h k w"))
                        nc.sync.dma_start(
                            out=t_t[:, :kn, :],
                            in_=tp[b, k0:k0 + kn, 0:128, :].rearrange("k h w -> h k w"))
                    else:
                        for bi in range(2):
                            hs = slice(64 * bi, 64 * bi + 64)
                            nc.sync.dma_start(
                                out=s_t[hs, :kn, :],
                                in_=sp[bb[bi], k0:k0 + kn, 128:192, :].rearrange(
                                    "k h w -> h k w"))
                            nc.sync.dma_start(
                                out=t_t[hs, :kn, :],
                                in_=tp[bb[bi], k0:k0 + kn, 128:192, :].rearrange(
                                    "k h w -> h k w"))

                    s = _slot(pair, ci, gi)
                    if en_stream:
                        j1 = jpool.tile([128, 6, W], F32, tag="junk")
                        nc.scalar.activation(
                            out=j1[:, :kn, :], in_=s_t[:, :kn, :], func=AF.Exp,
                            scale=1.0 / TEMP,
                            accum_out=stats_act[:, _c_zs(s):_c_zs(s) + 1])
                        et_t = epool.tile([128, 6, W], F32, tag="et")
                        nc.scalar.activation(
                            out=et_t[:, :kn, :], in_=t_t[:, :kn, :], func=AF.Exp,
                            scale=1.0 / TEMP,
                            accum_out=stats_act[:, _c_zt(s):_c_zt(s) + 1])
                        j2 = jpool.tile([128, 6, W], F32, tag="junk")
                        if s2_engine == "gpsimd":
                            nc.gpsimd.scalar_tensor_tensor(
                                out=j2[:, :kn, :], in0=s_t[:, :kn, :], scalar=1.0,
                                in1=s_t[:, :kn, :], op0=ALU.mult, op1=ALU.mult,
                                accum_out=stats_gp[:, _c_s2(s):_c_s2(s) + 1])
                        else:
                            nc.scalar.activation(
                                out=j2[:, :kn, :], in_=s_t[:, :kn, :],
                                func=AF.Square, scale=1.0,
                                accum_out=stats_gp[:, _c_s2(s):_c_s2(s) + 1])
                        j3 = jpool.tile([128, 6, W], F32, tag="junk")
                        nc.vector.scalar_tensor_tensor(
                            out=j3[:, :kn, :], in0=et_t[:, :kn, :], scalar=1.0,
                            in1=t_t[:, :kn, :], op0=ALU.mult, op1=ALU.mult,
                            accum_out=stats_dve[:, _c_a1(s):_c_a1(s) + 1])
                        j4 = jpool.tile([128, 6, W], F32, tag="junk")
                        nc.vector.scalar_tensor_tensor(
                            out=j4[:, :kn, :], in0=et_t[:, :kn, :], scalar=1.0,
                            in1=s_t[:, :kn, :], op0=ALU.mult, op1=ALU.mult,
                            accum_out=stats_dve[:, _c_a2(s):_c_a2(s) + 1])
                    if not en_m2:
                        continue

                    # M2 matmuls against the transposed gaussian-x factors.
                    # PSUM start=True zeroes the whole 2KB zero-region (the
                    # bank), so each psum tile gets exactly ONE group: start
                    # on the first matmul into the tile, stop on the last.
                    for kl in range(kn):
                        k = k0 + kl
                        for bi in range(2):
                            if gi < 2 and bi != gi:
                                continue
                            col = ((pair * 2 + bi) * K + k) * P
                            for jc, (j0, jn) in enumerate(((0, 128), (128, 64))):
                                if gi < 2:
                                    lhsT = s_t[:, kl, j0:j0 + jn]
                                    rhs = gxT1[:, col:col + P]
                                else:
                                    hs = slice(64 * bi, 64 * bi + 64)
                                    lhsT = s_t[hs, kl, j0:j0 + jn]
                                    rhs = gxT2r[hs, col:col + P]
                                nc.tensor.matmul(
                                    out=ps[(bi, jc)][:, k * P:(k + 1) * P],
                                    lhsT=lhsT, rhs=rhs,
                                    start=(gi < 2 and ci == 0 and kl == 0),
                                    stop=(gi == 2 and ci == 2 and kl == kn - 1),
                                    skip_group_check=True)

            for bi in range(2):
                if not en_m2:
                    continue
                bcols = slice((pair * 2 + bi) * K * P, (pair * 2 + bi + 1) * K * P)
                mj0 = mjp.tile([128, K * P], F32, tag="mj0")
                nc.vector.scalar_tensor_tensor(
                    out=mj0, in0=ps[(bi, 0)][:, :], scalar=1.0,
                    in1=gyT1[:, bcols], op0=ALU.mult, op1=ALU.mult,
                    accum_out=stats_dve[:, _c_m2(pair, bi, 0):_c_m2(pair, bi, 0) + 1])
                mj1 = mjp.tile([64, K * P], F32, tag="mj1")
                nc.vector.scalar_tensor_tensor(
                    out=mj1, in0=ps[(bi, 1)][:, :], scalar=1.0,
                    in1=gyT2[0:64, bcols], op0=ALU.mult, op1=ALU.mult,
                    accum_out=stats_dve[0:64, _c_m2(pair, bi, 1):_c_m2(pair, bi, 1) + 1])

        # ---- BCE over the seg logits ----
        if en_bce:
            segx = bpool.tile([128, SEG_F], F32, tag="segx")
            nc.sync.dma_start(
                out=segx,
                in_=sg[:, :, :].rearrange("b (hp hf) w -> (b hp) (hf w)", hf=6))
            mkt = bpool.tile([128, SEG_F], F32, tag="mkt")
            nc.sync.dma_start(
                out=mkt,
                in_=mk[:, :, :].rearrange("b (hp hf) w -> (b hp) (hf w)", hf=6))
            ej = bpool.tile([128, SEG_F], F32, tag="ej")
            nc.scalar.activation(out=ej, in_=segx, func=AF.Exp, scale=1.0)
            lj = bpool.tile([128, SEG_F], F32, tag="lj")
            nc.scalar.activation(out=lj, in_=ej, func=AF.Ln, bias=1.0, scale=1.0,
                                 accum_out=stats_act[:, C_SP:C_SP + 1])
            xj = bpool.tile([128, SEG_F], F32, tag="xj")
            nc.vector.scalar_tensor_tensor(
                out=xj, in0=segx, scalar=1.0, in1=mkt,
                op0=ALU.mult, op1=ALU.mult,
                accum_out=stats_dve[:, C_XM:C_XM + 1])

        # ---- write partials ----
        nc.sync.dma_start(out=out_d[:, 0:64], in_=stats_act[:, :])
        nc.sync.dma_start(out=out_d[:, 64:128], in_=stats_dve[:, :])
        nc.sync.dma_start(out=out_d[:, 128:160], in_=stats_gp[:, :])

    nc.compile()
    return nc


_NC_CACHE = {}


def _get_nc(s2_engine="act"):
    if s2_engine not in _NC_CACHE:
        _NC_CACHE[s2_engine] = build_nc(s2_engine)
    return _NC_CACHE[s2_engine]


def host_prep_core(keypoints, visibilities):
    """Per-core host preprocessing from the tiny keypoint tensors.

    Returns (bx[128,KP_TILES], by[128,KP_TILES], T2[BPC] float64, denom[BPC]).
    Matches reference semantics exactly: x = floor(f32(kx * 191)),
    valid = (vis > 0) & (0 <= x < W) & (0 <= y < H). gx carries the valid
    mask (via bias = 1e9 so exp underflows to exactly 0), gy does not.
    """
    kx = keypoints[..., 0].astype(np.float32) * np.float32(W - 1)
    ky = keypoints[..., 1].astype(np.float32) * np.float32(H - 1)
    x = np.floor(kx)
    y = np.floor(ky)
    valid = ((visibilities > 0) & (x >= 0) & (x < W) & (y >= 0) & (y < H))

    # bias rows ordered (b, k, p) to match the gaussian column order
    bx = np.full(KP_TILES * 128, 1e9, dtype=np.float32)
    by = np.full(KP_TILES * 128, 1e9, dtype=np.float32)
    xr = np.transpose(x, (0, 2, 1)).reshape(-1)       # [b,k,p] flat
    yr = np.transpose(y, (0, 2, 1)).reshape(-1)
    vr = np.transpose(valid, (0, 2, 1)).reshape(-1)
    bx[:KP] = np.where(vr, -xr, np.float32(1e9))
    by[:KP] = -yr  # gy has no valid mask in the reference
    bx = bx.reshape(KP_TILES, 128).T.copy()           # [128, KP_TILES]
    by = by.reshape(KP_TILES, 128).T.copy()

    # T2 = sum over target^2, in float64 on host (keypoints-only quantity)
    ax = np.arange(W, dtype=np.float64)
    gx = np.exp(-((ax[None, None, None, :] - x[..., None].astype(np.float64)) ** 2)
                * INV2S2) * valid[..., None]          # [BPC,P,K,W]
    gy = np.exp(-((ax[None, None, None, :] - y[..., None].astype(np.float64)) ** 2)
                * INV2S2)                             # [BPC,P,K,H]
    gxg = np.einsum("bpki,bqki->bkpq", gx, gx)
    gyg = np.einsum("bpkj,bqkj->bkpq", gy, gy)
    T2 = np.einsum("bkpq,bkpq->b", gxg, gyg)

    denom = visibilities.sum(axis=(1, 2)).astype(np.float64) + 1e-6
    return bx, by, T2, denom


def core_sample_stats(pa, sloc):
    """Extract per-sample scalar stats from one core's [128, cols] partials."""
    pa = pa.astype(np.float64)
    act, dve, gp = pa[:, 0:64], pa[:, 64:128], pa[:, 128:160]
    pair, bi = sloc // 2, sloc % 2
    Zs = Zt = A1 = A2 = S2 = 0.0
    for ci in range(3):
        sA = _slot(pair, ci, bi)     # own h<128 group: all partitions
        sB = _slot(pair, ci, 2)      # packed h>=128 group: own half
        hp = slice(64 * bi, 64 * bi + 64)
        Zs += act[:, _c_zs(sA)].sum() + act[hp, _c_zs(sB)].sum()
        Zt += act[:, _c_zt(sA)].sum() + act[hp, _c_zt(sB)].sum()
        A1 += dve[:, _c_a1(sA)].sum() + dve[hp, _c_a1(sB)].sum()
        A2 += dve[:, _c_a2(sA)].sum() + dve[hp, _c_a2(sB)].sum()
        S2 += gp[:, _c_s2(sA)].sum() + gp[hp, _c_s2(sB)].sum()
    M2 = (dve[:, _c_m2(pair, bi, 0)].sum()
          + dve[0:64, _c_m2(pair, bi, 1)].sum())
    return dict(Zs=Zs, Zt=Zt, A1=A1, A2=A2, S2=S2, M2=M2)


def host_reduce(partials_list, T2_list, denom_list):
    """Combine per-core [128, PARTIALS_COLS] partials into the final loss."""
    kl_sum = 0.0
    sp_sum = 0.0
    xm_sum = 0.0
    pose_terms = []
    for c in range(NCORES):
        pa = partials_list[c].astype(np.float64)
        sp_sum += pa[:, C_SP].sum()
        xm_sum += pa[:, 64 + C_XM].sum()
        for sloc in range(BPC):
            st = core_sample_stats(partials_list[c], sloc)
            kl_sum += ((st["A1"] - st["A2"]) / (TEMP * st["Zt"])
                       - np.log(st["Zt"]) + np.log(st["Zs"]))
            sse = st["S2"] - 2.0 * st["M2"] + T2_list[c][sloc]
            pose_terms.append(sse / denom_list[c][sloc])

    pose_distill = (TEMP ** 2) * kl_sum / B
    task_seg = (sp_sum - xm_sum) / (B * H * W)
    task_pose = float(np.mean(pose_terms))
    total = ALPHA * pose_distill + (1.0 - ALPHA) * (task_seg + task_pose)
    return np.float32(total)


def make_in_maps(s_seg_logits, s_pose_logits, t_pose_logits, mask,
                 keypoints, visibilities):
    in_maps, T2s, denoms = [], [], []
    for c in range(NCORES):
        sl = slice(BPC * c, BPC * (c + 1))
        bx, by, T2, denom = host_prep_core(keypoints[sl], visibilities[sl])
        in_maps.append({
            "s_pose": np.ascontiguousarray(s_pose_logits[sl]),
            "t_pose": np.ascontiguousarray(t_pose_logits[sl]),
            "s_seg": np.ascontiguousarray(s_seg_logits[sl, 0]),
            "mask": np.ascontiguousarray(mask[sl]),
            "bx": bx,
            "by": by,
        })
        T2s.append(T2)
        denoms.append(denom)
    return in_maps, T2s, denoms


def kernel(s_seg_logits, s_pose_logits, t_seg_logits, t_pose_logits,
           mask, keypoints, visibilities):
    nc = _get_nc()
    in_maps, T2s, denoms = make_in_maps(
        s_seg_logits, s_pose_logits, t_pose_logits, mask,
        keypoints, visibilities)
    res = run_bass_kernel_spmd(nc, in_maps, core_ids=list(range(NCORES)))
    partials = [r["partials"] for r in res.results]
    return host_reduce(partials, T2s, denoms)


# revision 21
# speedup vs baseline: 1.2474x; 1.2474x over previous
"""Trainium2 Bass kernel for the DistillationLoss problem.

Strategy (data parallel over batch, 8 cores x 4 samples):
  total = ALPHA*distill + (1-ALPHA)*(task_seg + task_pose)

  * seg_distill is identically 0 (softmax over a single channel), so
    t_seg_logits is never read.
  * pose_distill per sample reduces to scalars computed in one streaming
    pass over s_pose/t_pose:
        Zs = sum exp(s/T), Zt = sum exp(t/T),
        A1 = sum exp(t/T)*t, A2 = sum exp(t/T)*s
        KL_b = (A1-A2)/(T*Zt) - ln Zt + ln Zs
    (logits ~ N(0,1) so exp without max-subtraction is safe in fp32)
  * keypoints MSE per sample decomposes as S2 - 2*M2 + T2 with
        S2 = sum s^2                      (device, streaming pass)
        M2 = sum_{p} gx_p^T S gy_p        (device, PE matmuls vs transposed
                                           gaussian factors; avoids ever
                                           materializing the target heatmaps)
        T2 = sum tg^2                     (host, tiny: function of keypoints only)
  * BCE uses softplus(x) - x*m; softplus via Ln(exp(x)+1) on ACT.

Device returns per-partition partial sums; the host reduces in float64.
"""

import numpy as np
from contextlib import ExitStack

import concourse.bass as bass
import concourse.bacc as bacc
import concourse.tile as tile
from concourse import mybir
from concourse.bass_utils import run_bass_kernel_spmd

F32 = mybir.dt.float32
AF = mybir.ActivationFunctionType
ALU = mybir.AluOpType

B, P, K, H, W = 32, 8, 17, 192, 192
ALPHA, TEMP, SIGMA = 0.5, 2.0, 3.0
INV2S2 = 1.0 / (2.0 * SIGMA * SIGMA)
NCORES = 8
BPC = B // NCORES            # samples per core (4)
NPAIR = BPC // 2             # sample pairs per core (2)
KP = BPC * K * P             # gaussian rows per core (544)
KP_TILES = (KP + 127) // 128  # 5
KCH = [(0, 6), (6, 6), (12, 5)]  # k-chunks of the K=17 axis
SEG_F = BPC * H * W // 128   # free dim of flattened seg tiles (1152)

# ---- stat column maps (shared by device builder and host reducer) ----
# stats_act[128, 64]: per (pair, chunk, grp) slot: ZS, ZT ; then SP
# stats_dve[128, 64]: per slot: A1, A2 ; then 8x M2 ; then XM
# stats_gp [128, 32]: per slot: S2
NSLOT = NPAIR * 3 * 3


def _slot(pair, chunk, grp):
    return (pair * 3 + chunk) * 3 + grp


def _c_zs(s): return 2 * s
def _c_zt(s): return 2 * s + 1
C_SP = 2 * NSLOT            # 36
def _c_a1(s): return 2 * s
def _c_a2(s): return 2 * s + 1
def _c_m2(pair, bi, jc): return 2 * NSLOT + (pair * 2 + bi) * 2 + jc  # 36..43
C_XM = 2 * NSLOT + NPAIR * 2 * 2            # 36 + 8 = 44
def _c_s2(s): return s

PARTIALS_COLS = 160  # [0:64] act, [64:128] dve, [128:160] gp


def build_nc(s2_engine="act", en_stream=True, en_m2=True, en_bce=True,
             s2_dve_mod=1, bce_first=False, bufs_st=4, bufs_e=2, bufs_j=3,
             kch=((0, 9), (9, 8))):
    nc = bacc.Bacc("TRN2", target_bir_lowering=False)

    sp = nc.dram_tensor("s_pose", [BPC, K, H, W], F32, kind="ExternalInput")
    tp = nc.dram_tensor("t_pose", [BPC, K, H, W], F32, kind="ExternalInput")
    sg = nc.dram_tensor("s_seg", [BPC, H, W], F32, kind="ExternalInput")
    mk = nc.dram_tensor("mask", [BPC, H, W], F32, kind="ExternalInput")
    bxd = nc.dram_tensor("bx", [128, KP_TILES], F32, kind="ExternalInput")
    byd = nc.dram_tensor("by", [128, KP_TILES], F32, kind="ExternalInput")
    out_d = nc.dram_tensor("partials", [128, PARTIALS_COLS], F32, kind="ExternalOutput")

    iota_c = nc.inline_tensor(
        np.tile(np.arange(W, dtype=np.float32), (128, 1)), name="iota_c")
    ident_c = nc.inline_tensor(np.eye(128, dtype=np.float32), name="ident_c")

    with tile.TileContext(nc) as tc, ExitStack() as ctx:
        const = ctx.enter_context(tc.tile_pool(name="const", bufs=1))
        gnat = ctx.enter_context(tc.tile_pool(name="gnat", bufs=4))
        tps = ctx.enter_context(tc.tile_pool(name="tps", bufs=2, space="PSUM"))
        spool = ctx.enter_context(tc.tile_pool(name="spool", bufs=bufs_st))
        tpool = ctx.enter_context(tc.tile_pool(name="tpool", bufs=bufs_st))
        epool = ctx.enter_context(tc.tile_pool(name="epool", bufs=bufs_e))
        jpool = ctx.enter_context(tc.tile_pool(name="jpool", bufs=bufs_j))
        dpool = ctx.enter_context(tc.tile_pool(name="dpool", bufs=bufs_e))
        m2ps = ctx.enter_context(tc.tile_pool(name="m2ps", bufs=2, space="PSUM"))
        mjp = ctx.enter_context(tc.tile_pool(name="mjp", bufs=2))
        bpool = ctx.enter_context(tc.tile_pool(name="bpool", bufs=2))

        # ---- constants ----
        iota_t = const.tile([128, W], F32)
        nc.sync.dma_start(out=iota_t, in_=iota_c[:, :])
        ident_t = const.tile([128, 128], F32)
        nc.sync.dma_start(out=ident_t, in_=ident_c[:, :])
        bx_t = const.tile([128, KP_TILES], F32)
        nc.sync.dma_start(out=bx_t, in_=bxd[:, :])
        by_t = const.tile([128, KP_TILES], F32)
        nc.sync.dma_start(out=by_t, in_=byd[:, :])

        # ---- stats tiles ----
        stats_act = const.tile([128, 64], F32)
        stats_dve = const.tile([128, 64], F32)
        stats_gp = const.tile([128, 32], F32)
        nc.gpsimd.memset(stats_act, 0.0)
        nc.gpsimd.memset(stats_dve, 0.0)
        nc.gpsimd.memset(stats_gp, 0.0)

        # ---- gaussian factors, transposed: g[row, col] -> gT[coord, row] ----
        # gxT1 [128,KP]: coords h 0:128 ; gxT2r [128,KP]: h 128:192 in parts
        # 0:64 AND replicated into parts 64:128 (so both halves of packed B01
        # tiles find a partition-aligned rhs).
        gxT1 = const.tile([128, KP], F32)
        gxT2r = const.tile([128, KP], F32)
        gyT1 = const.tile([128, KP], F32)
        gyT2 = const.tile([64, KP], F32)

        for (bias_t, gT1, gT2, repl) in ((bx_t, gxT1, gxT2r, True),
                                         (by_t, gyT1, gyT2, False)):
            for t in range(KP_TILES):
                sz = min(128, KP - t * 128)
                off = t * 128
                gsq = gnat.tile([128, W], F32, tag="gsq")
                nc.scalar.activation(out=gsq[:sz], in_=iota_t[:sz],
                                     func=AF.Square, bias=bias_t[:sz, t:t + 1],
                                     scale=1.0)
                gex = gnat.tile([128, W], F32, tag="gex")
                nc.scalar.activation(out=gex[:sz], in_=gsq[:sz],
                                     func=AF.Exp, scale=-INV2S2)
                pt1 = tps.tile([128, 128], F32, tag="pt1")
                nc.tensor.transpose(out=pt1[:128, :sz], in_=gex[:sz, 0:128],
                                    identity=ident_t[:sz, :sz])
                nc.scalar.copy(out=gT1[:, off:off + sz], in_=pt1[:128, :sz])
                pt2 = tps.tile([128, 128], F32, tag="pt2")
                nc.tensor.transpose(out=pt2[:64, :sz], in_=gex[:sz, 128:192],
                                    identity=ident_t[:sz, :sz])
                nc.scalar.copy(out=gT2[0:64, off:off + sz], in_=pt2[:64, :sz])
            if repl:
                nc.sync.dma_start(out=gT2[64:128, 0:KP], in_=gT2[0:64, 0:KP])

        # ---- BCE over the seg logits ----
        def emit_bce():
            segx = bpool.tile([128, SEG_F], F32, tag="segx")
            nc.sync.dma_start(
                out=segx,
                in_=sg[:, :, :].rearrange("b (hp hf) w -> (b hp) (hf w)", hf=6))
            mkt = bpool.tile([128, SEG_F], F32, tag="mkt")
            nc.sync.dma_start(
                out=mkt,
                in_=mk[:, :, :].rearrange("b (hp hf) w -> (b hp) (hf w)", hf=6))
            ej = bpool.tile([128, SEG_F], F32, tag="ej")
            nc.scalar.activation(out=ej, in_=segx, func=AF.Exp, scale=1.0)
            lj = bpool.tile([128, SEG_F], F32, tag="lj")
            nc.scalar.activation(out=lj, in_=ej, func=AF.Ln, bias=1.0, scale=1.0,
                                 accum_out=stats_act[:, C_SP:C_SP + 1])
            xj = bpool.tile([128, SEG_F], F32, tag="xj")
            nc.vector.scalar_tensor_tensor(
                out=xj, in0=segx, scalar=1.0, in1=mkt,
                op0=ALU.mult, op1=ALU.mult,
                accum_out=stats_dve[:, C_XM:C_XM + 1])

        kch_l = KCH if kch is None else list(kch)
        mx = max(kn for _, kn in kch_l)
        if en_bce and bce_first:
            emit_bce()
        # ---- streaming pass + M2 matmuls ----
        for pair in range(NPAIR):
            bb = (2 * pair, 2 * pair + 1)
            ps = {}
            for bi in range(2):
                ps[(bi, 0)] = m2ps.tile([128, K * P], F32, tag="psj0",
                                        name=f"ps{pair}_{bi}_0")
                ps[(bi, 1)] = m2ps.tile([64, K * P], F32, tag="psj1",
                                        name=f"ps{pair}_{bi}_1")

            for ci, (k0, kn) in enumerate(kch_l):
                for gi in range(3):  # 0: b0 h<128, 1: b1 h<128, 2: packed h>=128
                    s_t = spool.tile([128, mx, W], F32, tag="s")
                    t_t = tpool.tile([128, mx, W], F32, tag="t")
                    if gi < 2:
                        b = bb[gi]
                        nc.sync.dma_start(
                            out=s_t[:, :kn, :],
                            in_=sp[b, k0:k0 + kn, 0:128, :].rearrange("k h w -> h k w"))
                        nc.sync.dma_start(
                            out=t_t[:, :kn, :],
                            in_=tp[b, k0:k0 + kn, 0:128, :].rearrange("k h w -> h k w"))
                    else:
                        for bi in range(2):
                            hs = slice(64 * bi, 64 * bi + 64)
                            nc.sync.dma_start(
                                out=s_t[hs, :kn, :],
                                in_=sp[bb[bi], k0:k0 + kn, 128:192, :].rearrange(
                                    "k h w -> h k w"))
                            nc.sync.dma_start(
                                out=t_t[hs, :kn, :],
                                in_=tp[bb[bi], k0:k0 + kn, 128:192, :].rearrange(
                                    "k h w -> h k w"))

                    s = _slot(pair, ci, gi)
                    if en_stream:
                        j1 = jpool.tile([128, mx, W], F32, tag="junk")
                        nc.scalar.activation(
                            out=j1[:, :kn, :], in_=s_t[:, :kn, :], func=AF.Exp,
                            scale=1.0 / TEMP,
                            accum_out=stats_act[:, _c_zs(s):_c_zs(s) + 1])
                        et_t = epool.tile([128, mx, W], F32, tag="et")
                        nc.scalar.activation(
                            out=et_t[:, :kn, :], in_=t_t[:, :kn, :], func=AF.Exp,
                            scale=1.0 / TEMP,
                            accum_out=stats_act[:, _c_zt(s):_c_zt(s) + 1])
                        j2 = jpool.tile([128, mx, W], F32, tag="junk")
                        if s2_dve_mod and s % s2_dve_mod == 0:
                            nc.vector.scalar_tensor_tensor(
                                out=j2[:, :kn, :], in0=s_t[:, :kn, :], scalar=1.0,
                                in1=s_t[:, :kn, :], op0=ALU.mult, op1=ALU.mult,
                                accum_out=stats_gp[:, _c_s2(s):_c_s2(s) + 1])
                        else:
                            nc.scalar.activation(
                                out=j2[:, :kn, :], in_=s_t[:, :kn, :],
                                func=AF.Square, scale=1.0,
                                accum_out=stats_gp[:, _c_s2(s):_c_s2(s) + 1])
                        d_t = dpool.tile([128, mx, W], F32, tag="d")
                        nc.gpsimd.tensor_tensor(
                            out=d_t[:, :kn, :], in0=t_t[:, :kn, :],
                            in1=s_t[:, :kn, :], op=ALU.subtract)
                        j3 = jpool.tile([128, mx, W], F32, tag="junk")
                        nc.vector.scalar_tensor_tensor(
                            out=j3[:, :kn, :], in0=et_t[:, :kn, :], scalar=1.0,
                            in1=d_t[:, :kn, :], op0=ALU.mult, op1=ALU.mult,
                            accum_out=stats_dve[:, _c_a1(s):_c_a1(s) + 1])
                    if not en_m2:
                        continue

                    # M2 matmuls against the transposed gaussian-x factors.
                    # PSUM start=True zeroes the whole 2KB zero-region (the
                    # bank), so each psum tile gets exactly ONE group: start
                    # on the first matmul into the tile, stop on the last.
                    for kl in range(kn):
                        k = k0 + kl
                        for bi in range(2):
                            if gi < 2 and bi != gi:
                                continue
                            col = ((pair * 2 + bi) * K + k) * P
                            for jc, (j0, jn) in enumerate(((0, 128), (128, 64))):
                                if gi < 2:
                                    lhsT = s_t[:, kl, j0:j0 + jn]
                                    rhs = gxT1[:, col:col + P]
                                else:
                                    hs = slice(64 * bi, 64 * bi + 64)
                                    lhsT = s_t[hs, kl, j0:j0 + jn]
                                    rhs = gxT2r[hs, col:col + P]
                                nc.tensor.matmul(
                                    out=ps[(bi, jc)][:, k * P:(k + 1) * P],
                                    lhsT=lhsT, rhs=rhs,
                                    start=(gi < 2 and ci == 0 and kl == 0),
                                    stop=(gi == 2 and ci == 2 and kl == kn - 1),
                                    skip_group_check=True)

            for bi in range(2):
                if not en_m2:
                    continue
                bcols = slice((pair * 2 + bi) * K * P, (pair * 2 + bi + 1) * K * P)
                mj0 = mjp.tile([128, K * P], F32, tag="mj0")
                nc.vector.scalar_tensor_tensor(
                    out=mj0, in0=ps[(bi, 0)][:, :], scalar=1.0,
                    in1=gyT1[:, bcols], op0=ALU.mult, op1=ALU.mult,
                    accum_out=stats_dve[:, _c_m2(pair, bi, 0):_c_m2(pair, bi, 0) + 1])
                mj1 = mjp.tile([64, K * P], F32, tag="mj1")
                nc.vector.scalar_tensor_tensor(
                    out=mj1, in0=ps[(bi, 1)][:, :], scalar=1.0,
                    in1=gyT2[0:64, bcols], op0=ALU.mult, op1=ALU.mult,
                    accum_out=stats_dve[0:64, _c_m2(pair, bi, 1):_c_m2(pair, bi, 1) + 1])


        if en_bce and not bce_first:
            emit_bce()

        # ---- write partials ----
        nc.sync.dma_start(out=out_d[:, 0:64], in_=stats_act[:, :])
        nc.sync.dma_start(out=out_d[:, 64:128], in_=stats_dve[:, :])
        nc.sync.dma_start(out=out_d[:, 128:160], in_=stats_gp[:, :])

    nc.compile()
    return nc


_NC_CACHE = {}


def _get_nc(s2_engine="act"):
    if s2_engine not in _NC_CACHE:
        _NC_CACHE[s2_engine] = build_nc(s2_engine)
    return _NC_CACHE[s2_engine]


def host_prep_core(keypoints, visibilities):
    """Per-core host preprocessing from the tiny keypoint tensors.

    Returns (bx[128,KP_TILES], by[128,KP_TILES], T2[BPC] float64, denom[BPC]).
    Matches reference semantics exactly: x = floor(f32(kx * 191)),
    valid = (vis > 0) & (0 <= x < W) & (0 <= y < H). gx carries the valid
    mask (via bias = 1e9 so exp underflows to exactly 0), gy does not.
    """
    kx = keypoints[..., 0].astype(np.float32) * np.float32(W - 1)
    ky = keypoints[..., 1].astype(np.float32) * np.float32(H - 1)
    x = np.floor(kx)
    y = np.floor(ky)
    valid = ((visibilities > 0) & (x >= 0) & (x < W) & (y >= 0) & (y < H))

    # bias rows ordered (b, k, p) to match the gaussian column order
    bx = np.full(KP_TILES * 128, 1e9, dtype=np.float32)
    by = np.full(KP_TILES * 128, 1e9, dtype=np.float32)
    xr = np.transpose(x, (0, 2, 1)).reshape(-1)       # [b,k,p] flat
    yr = np.transpose(y, (0, 2, 1)).reshape(-1)
    vr = np.transpose(valid, (0, 2, 1)).reshape(-1)
    bx[:KP] = np.where(vr, -xr, np.float32(1e9))
    by[:KP] = -yr  # gy has no valid mask in the reference
    bx = bx.reshape(KP_TILES, 128).T.copy()           # [128, KP_TILES]
    by = by.reshape(KP_TILES, 128).T.copy()

    # T2 = sum over target^2, in float64 on host (keypoints-only quantity)
    ax = np.arange(W, dtype=np.float64)
    gx = np.exp(-((ax[None, None, None, :] - x[..., None].astype(np.float64)) ** 2)
                * INV2S2) * valid[..., None]          # [BPC,P,K,W]
    gy = np.exp(-((ax[None, None, None, :] - y[..., None].astype(np.float64)) ** 2)
                * INV2S2)                             # [BPC,P,K,H]
    gxg = np.einsum("bpki,bqki->bkpq", gx, gx)
    gyg = np.einsum("bpkj,bqkj->bkpq", gy, gy)
    T2 = np.einsum("bkpq,bkpq->b", gxg, gyg)

    denom = visibilities.sum(axis=(1, 2)).astype(np.float64) + 1e-6
    return bx, by, T2, denom


def core_sample_stats(pa, sloc):
    """Extract per-sample scalar stats from one core's [128, cols] partials."""
    pa = pa.astype(np.float64)
    act, dve, gp = pa[:, 0:64], pa[:, 64:128], pa[:, 128:160]
    pair, bi = sloc // 2, sloc % 2
    Zs = Zt = A1 = A2 = S2 = 0.0
    for ci in range(3):
        sA = _slot(pair, ci, bi)     # own h<128 group: all partitions
        sB = _slot(pair, ci, 2)      # packed h>=128 group: own half
        hp = slice(64 * bi, 64 * bi + 64)
        Zs += act[:, _c_zs(sA)].sum() + act[hp, _c_zs(sB)].sum()
        Zt += act[:, _c_zt(sA)].sum() + act[hp, _c_zt(sB)].sum()
        A1 += dve[:, _c_a1(sA)].sum() + dve[hp, _c_a1(sB)].sum()
        A2 += dve[:, _c_a2(sA)].sum() + dve[hp, _c_a2(sB)].sum()
        S2 += gp[:, _c_s2(sA)].sum() + gp[hp, _c_s2(sB)].sum()
    M2 = (dve[:, _c_m2(pair, bi, 0)].sum()
          + dve[0:64, _c_m2(pair, bi, 1)].sum())
    return dict(Zs=Zs, Zt=Zt, A1=A1, A2=A2, S2=S2, M2=M2)


def host_reduce(partials_list, T2_list, denom_list):
    """Combine per-core [128, PARTIALS_COLS] partials into the final loss."""
    kl_sum = 0.0
    sp_sum = 0.0
    xm_sum = 0.0
    pose_terms = []
    for c in range(NCORES):
        pa = partials_list[c].astype(np.float64)
        sp_sum += pa[:, C_SP].sum()
        xm_sum += pa[:, 64 + C_XM].sum()
        for sloc in range(BPC):
            st = core_sample_stats(partials_list[c], sloc)
            kl_sum += (st["A1"] / (TEMP * st["Zt"])
                       - np.log(st["Zt"]) + np.log(st["Zs"]))
            sse = st["S2"] - 2.0 * st["M2"] + T2_list[c][sloc]
            pose_terms.append(sse / denom_list[c][sloc])

    pose_distill = (TEMP ** 2) * kl_sum / B
    task_seg = (sp_sum - xm_sum) / (B * H * W)
    task_pose = float(np.mean(pose_terms))
    total = ALPHA * pose_distill + (1.0 - ALPHA) * (task_seg + task_pose)
    return np.float32(total)


def make_in_maps(s_seg_logits, s_pose_logits, t_pose_logits, mask,
                 keypoints, visibilities):
    in_maps, T2s, denoms = [], [], []
    for c in range(NCORES):
        sl = slice(BPC * c, BPC * (c + 1))
        bx, by, T2, denom = host_prep_core(keypoints[sl], visibilities[sl])
        in_maps.append({
            "s_pose": np.ascontiguousarray(s_pose_logits[sl]),
            "t_pose": np.ascontiguousarray(t_pose_logits[sl]),
            "s_seg": np.ascontiguousarray(s_seg_logits[sl, 0]),
            "mask": np.ascontiguousarray(mask[sl]),
            "bx": bx,
            "by": by,
        })
        T2s.append(T2)
        denoms.append(denom)
    return in_maps, T2s, denoms


def kernel(s_seg_logits, s_pose_logits, t_seg_logits, t_pose_logits,
           mask, keypoints, visibilities):
    nc = _get_nc()
    in_maps, T2s, denoms = make_in_maps(
        s_seg_logits, s_pose_logits, t_pose_logits, mask,
        keypoints, visibilities)
    res = run_bass_kernel_spmd(nc, in_maps, core_ids=list(range(NCORES)))
    partials = [r["partials"] for r in res.results]
    return host_reduce(partials, T2s, denoms)


# revision 22
# speedup vs baseline: 1.2508x; 1.0027x over previous
"""Trainium2 Bass kernel for the DistillationLoss problem.

Strategy (data parallel over batch, 8 cores x 4 samples):
  total = ALPHA*distill + (1-ALPHA)*(task_seg + task_pose)

  * seg_distill is identically 0 (softmax over a single channel), so
    t_seg_logits is never read.
  * pose_distill per sample reduces to scalars computed in one streaming
    pass over s_pose/t_pose:
        Zs = sum exp(s/T), Zt = sum exp(t/T),
        A1 = sum exp(t/T)*t, A2 = sum exp(t/T)*s
        KL_b = (A1-A2)/(T*Zt) - ln Zt + ln Zs
    (logits ~ N(0,1) so exp without max-subtraction is safe in fp32)
  * keypoints MSE per sample decomposes as S2 - 2*M2 + T2 with
        S2 = sum s^2                      (device, streaming pass)
        M2 = sum_{p} gx_p^T S gy_p        (device, PE matmuls vs transposed
                                           gaussian factors; avoids ever
                                           materializing the target heatmaps)
        T2 = sum tg^2                     (host, tiny: function of keypoints only)
  * BCE uses softplus(x) - x*m; softplus via Ln(exp(x)+1) on ACT.

Device returns per-partition partial sums; the host reduces in float64.
"""

import numpy as np
from contextlib import ExitStack

import concourse.bass as bass
import concourse.bacc as bacc
import concourse.tile as tile
from concourse import mybir
from concourse.bass_utils import run_bass_kernel_spmd

F32 = mybir.dt.float32
AF = mybir.ActivationFunctionType
ALU = mybir.AluOpType

B, P, K, H, W = 32, 8, 17, 192, 192
ALPHA, TEMP, SIGMA = 0.5, 2.0, 3.0
INV2S2 = 1.0 / (2.0 * SIGMA * SIGMA)
NCORES = 8
BPC = B // NCORES            # samples per core (4)
NPAIR = BPC // 2             # sample pairs per core (2)
KP = BPC * K * P             # gaussian rows per core (544)
KP_TILES = (KP + 127) // 128  # 5
KCH = [(0, 6), (6, 6), (12, 5)]  # k-chunks of the K=17 axis
SEG_F = BPC * H * W // 128   # free dim of flattened seg tiles (1152)

# ---- stat column maps (shared by device builder and host reducer) ----
# stats_act[128, 64]: per (pair, chunk, grp) slot: ZS, ZT ; then SP
# stats_dve[128, 64]: per slot: A1, A2 ; then 8x M2 ; then XM
# stats_gp [128, 32]: per slot: S2
NSLOT = NPAIR * 3 * 3


def _slot(pair, chunk, grp):
    return (pair * 3 + chunk) * 3 + grp


def _c_zs(s): return 2 * s
def _c_zt(s): return 2 * s + 1
C_SP = 2 * NSLOT            # 36
def _c_a1(s): return 2 * s
def _c_a2(s): return 2 * s + 1
def _c_m2(pair, bi, jc): return 2 * NSLOT + (pair * 2 + bi) * 2 + jc  # 36..43
C_XM = 2 * NSLOT + NPAIR * 2 * 2            # 36 + 8 = 44
def _c_s2(s): return s

PARTIALS_COLS = 160  # [0:64] act, [64:128] dve, [128:160] gp


def build_nc(s2_engine="act", en_stream=True, en_m2=True, en_bce=True,
             s2_dve_mod=1, bce_first=False, bufs_st=4, bufs_e=2, bufs_j=3,
             kch=((0, 10), (10, 7))):
    nc = bacc.Bacc("TRN2", target_bir_lowering=False)

    sp = nc.dram_tensor("s_pose", [BPC, K, H, W], F32, kind="ExternalInput")
    tp = nc.dram_tensor("t_pose", [BPC, K, H, W], F32, kind="ExternalInput")
    sg = nc.dram_tensor("s_seg", [BPC, H, W], F32, kind="ExternalInput")
    mk = nc.dram_tensor("mask", [BPC, H, W], F32, kind="ExternalInput")
    bxd = nc.dram_tensor("bx", [128, KP_TILES], F32, kind="ExternalInput")
    byd = nc.dram_tensor("by", [128, KP_TILES], F32, kind="ExternalInput")
    out_d = nc.dram_tensor("partials", [128, PARTIALS_COLS], F32, kind="ExternalOutput")

    iota_c = nc.inline_tensor(
        np.tile(np.arange(W, dtype=np.float32), (128, 1)), name="iota_c")
    ident_c = nc.inline_tensor(np.eye(128, dtype=np.float32), name="ident_c")

    with tile.TileContext(nc) as tc, ExitStack() as ctx:
        const = ctx.enter_context(tc.tile_pool(name="const", bufs=1))
        gnat = ctx.enter_context(tc.tile_pool(name="gnat", bufs=4))
        tps = ctx.enter_context(tc.tile_pool(name="tps", bufs=2, space="PSUM"))
        spool = ctx.enter_context(tc.tile_pool(name="spool", bufs=bufs_st))
        tpool = ctx.enter_context(tc.tile_pool(name="tpool", bufs=bufs_st))
        epool = ctx.enter_context(tc.tile_pool(name="epool", bufs=bufs_e))
        jpool = ctx.enter_context(tc.tile_pool(name="jpool", bufs=bufs_j))
        dpool = ctx.enter_context(tc.tile_pool(name="dpool", bufs=bufs_e))
        m2ps = ctx.enter_context(tc.tile_pool(name="m2ps", bufs=2, space="PSUM"))
        mjp = ctx.enter_context(tc.tile_pool(name="mjp", bufs=2))
        bpool = ctx.enter_context(tc.tile_pool(name="bpool", bufs=2))

        # ---- constants ----
        iota_t = const.tile([128, W], F32)
        nc.sync.dma_start(out=iota_t, in_=iota_c[:, :])
        ident_t = const.tile([128, 128], F32)
        nc.sync.dma_start(out=ident_t, in_=ident_c[:, :])
        bx_t = const.tile([128, KP_TILES], F32)
        nc.sync.dma_start(out=bx_t, in_=bxd[:, :])
        by_t = const.tile([128, KP_TILES], F32)
        nc.sync.dma_start(out=by_t, in_=byd[:, :])

        # ---- stats tiles ----
        stats_act = const.tile([128, 64], F32)
        stats_dve = const.tile([128, 64], F32)
        stats_gp = const.tile([128, 32], F32)
        nc.gpsimd.memset(stats_act, 0.0)
        nc.gpsimd.memset(stats_dve, 0.0)
        nc.gpsimd.memset(stats_gp, 0.0)

        # ---- gaussian factors, transposed: g[row, col] -> gT[coord, row] ----
        # gxT1 [128,KP]: coords h 0:128 ; gxT2r [128,KP]: h 128:192 in parts
        # 0:64 AND replicated into parts 64:128 (so both halves of packed B01
        # tiles find a partition-aligned rhs).
        gxT1 = const.tile([128, KP], F32)
        gxT2r = const.tile([128, KP], F32)
        gyT1 = const.tile([128, KP], F32)
        gyT2 = const.tile([64, KP], F32)

        for (bias_t, gT1, gT2, repl) in ((bx_t, gxT1, gxT2r, True),
                                         (by_t, gyT1, gyT2, False)):
            for t in range(KP_TILES):
                sz = min(128, KP - t * 128)
                off = t * 128
                gsq = gnat.tile([128, W], F32, tag="gsq")
                nc.scalar.activation(out=gsq[:sz], in_=iota_t[:sz],
                                     func=AF.Square, bias=bias_t[:sz, t:t + 1],
                                     scale=1.0)
                gex = gnat.tile([128, W], F32, tag="gex")
                nc.scalar.activation(out=gex[:sz], in_=gsq[:sz],
                                     func=AF.Exp, scale=-INV2S2)
                pt1 = tps.tile([128, 128], F32, tag="pt1")
                nc.tensor.transpose(out=pt1[:128, :sz], in_=gex[:sz, 0:128],
                                    identity=ident_t[:sz, :sz])
                nc.scalar.copy(out=gT1[:, off:off + sz], in_=pt1[:128, :sz])
                pt2 = tps.tile([128, 128], F32, tag="pt2")
                nc.tensor.transpose(out=pt2[:64, :sz], in_=gex[:sz, 128:192],
                                    identity=ident_t[:sz, :sz])
                nc.scalar.copy(out=gT2[0:64, off:off + sz], in_=pt2[:64, :sz])
            if repl:
                nc.sync.dma_start(out=gT2[64:128, 0:KP], in_=gT2[0:64, 0:KP])

        # ---- BCE over the seg logits ----
        def emit_bce():
            segx = bpool.tile([128, SEG_F], F32, tag="segx")
            nc.sync.dma_start(
                out=segx,
                in_=sg[:, :, :].rearrange("b (hp hf) w -> (b hp) (hf w)", hf=6))
            mkt = bpool.tile([128, SEG_F], F32, tag="mkt")
            nc.sync.dma_start(
                out=mkt,
                in_=mk[:, :, :].rearrange("b (hp hf) w -> (b hp) (hf w)", hf=6))
            ej = bpool.tile([128, SEG_F], F32, tag="ej")
            nc.scalar.activation(out=ej, in_=segx, func=AF.Exp, scale=1.0)
            lj = bpool.tile([128, SEG_F], F32, tag="lj")
            nc.scalar.activation(out=lj, in_=ej, func=AF.Ln, bias=1.0, scale=1.0,
                                 accum_out=stats_act[:, C_SP:C_SP + 1])
            xj = bpool.tile([128, SEG_F], F32, tag="xj")
            nc.vector.scalar_tensor_tensor(
                out=xj, in0=segx, scalar=1.0, in1=mkt,
                op0=ALU.mult, op1=ALU.mult,
                accum_out=stats_dve[:, C_XM:C_XM + 1])

        kch_l = KCH if kch is None else list(kch)
        mx = max(kn for _, kn in kch_l)
        if en_bce and bce_first:
            emit_bce()
        # ---- streaming pass + M2 matmuls ----
        for pair in range(NPAIR):
            bb = (2 * pair, 2 * pair + 1)
            ps = {}
            for bi in range(2):
                ps[(bi, 0)] = m2ps.tile([128, K * P], F32, tag="psj0",
                                        name=f"ps{pair}_{bi}_0")
                ps[(bi, 1)] = m2ps.tile([64, K * P], F32, tag="psj1",
                                        name=f"ps{pair}_{bi}_1")

            for ci, (k0, kn) in enumerate(kch_l):
                for gi in range(3):  # 0: b0 h<128, 1: b1 h<128, 2: packed h>=128
                    s_t = spool.tile([128, mx, W], F32, tag="s")
                    t_t = tpool.tile([128, mx, W], F32, tag="t")
                    if gi < 2:
                        b = bb[gi]
                        nc.sync.dma_start(
                            out=s_t[:, :kn, :],
                            in_=sp[b, k0:k0 + kn, 0:128, :].rearrange("k h w -> h k w"))
                        nc.sync.dma_start(
                            out=t_t[:, :kn, :],
                            in_=tp[b, k0:k0 + kn, 0:128, :].rearrange("k h w -> h k w"))
                    else:
                        for bi in range(2):
                            hs = slice(64 * bi, 64 * bi + 64)
                            nc.sync.dma_start(
                                out=s_t[hs, :kn, :],
                                in_=sp[bb[bi], k0:k0 + kn, 128:192, :].rearrange(
                                    "k h w -> h k w"))
                            nc.sync.dma_start(
                                out=t_t[hs, :kn, :],
                                in_=tp[bb[bi], k0:k0 + kn, 128:192, :].rearrange(
                                    "k h w -> h k w"))

                    s = _slot(pair, ci, gi)
                    if en_stream:
                        j1 = jpool.tile([128, mx, W], F32, tag="junk")
                        nc.scalar.activation(
                            out=j1[:, :kn, :], in_=s_t[:, :kn, :], func=AF.Exp,
                            scale=1.0 / TEMP,
                            accum_out=stats_act[:, _c_zs(s):_c_zs(s) + 1])
                        et_t = epool.tile([128, mx, W], F32, tag="et")
                        nc.scalar.activation(
                            out=et_t[:, :kn, :], in_=t_t[:, :kn, :], func=AF.Exp,
                            scale=1.0 / TEMP,
                            accum_out=stats_act[:, _c_zt(s):_c_zt(s) + 1])
                        j2 = jpool.tile([128, mx, W], F32, tag="junk")
                        if s2_dve_mod and s % s2_dve_mod == 0:
                            nc.vector.scalar_tensor_tensor(
                                out=j2[:, :kn, :], in0=s_t[:, :kn, :], scalar=1.0,
                                in1=s_t[:, :kn, :], op0=ALU.mult, op1=ALU.mult,
                                accum_out=stats_gp[:, _c_s2(s):_c_s2(s) + 1])
                        else:
                            nc.scalar.activation(
                                out=j2[:, :kn, :], in_=s_t[:, :kn, :],
                                func=AF.Square, scale=1.0,
                                accum_out=stats_gp[:, _c_s2(s):_c_s2(s) + 1])
                        d_t = dpool.tile([128, mx, W], F32, tag="d")
                        nc.gpsimd.tensor_tensor(
                            out=d_t[:, :kn, :], in0=t_t[:, :kn, :],
                            in1=s_t[:, :kn, :], op=ALU.subtract)
                        j3 = jpool.tile([128, mx, W], F32, tag="junk")
                        nc.vector.scalar_tensor_tensor(
                            out=j3[:, :kn, :], in0=et_t[:, :kn, :], scalar=1.0,
                            in1=d_t[:, :kn, :], op0=ALU.mult, op1=ALU.mult,
                            accum_out=stats_dve[:, _c_a1(s):_c_a1(s) + 1])
                    if not en_m2:
                        continue

                    # M2 matmuls against the transposed gaussian-x factors.
                    # PSUM start=True zeroes the whole 2KB zero-region (the
                    # bank), so each psum tile gets exactly ONE group: start
                    # on the first matmul into the tile, stop on the last.
                    for kl in range(kn):
                        k = k0 + kl
                        for bi in range(2):
                            if gi < 2 and bi != gi:
                                continue
                            col = ((pair * 2 + bi) * K + k) * P
                            for jc, (j0, jn) in enumerate(((0, 128), (128, 64))):
                                if gi < 2:
                                    lhsT = s_t[:, kl, j0:j0 + jn]
                                    rhs = gxT1[:, col:col + P]
                                else:
                                    hs = slice(64 * bi, 64 * bi + 64)
                                    lhsT = s_t[hs, kl, j0:j0 + jn]
                                    rhs = gxT2r[hs, col:col + P]
                                nc.tensor.matmul(
                                    out=ps[(bi, jc)][:, k * P:(k + 1) * P],
                                    lhsT=lhsT, rhs=rhs,
                                    start=(gi < 2 and ci == 0 and kl == 0),
                                    stop=(gi == 2 and ci == 2 and kl == kn - 1),
                                    skip_group_check=True)

            for bi in range(2):
                if not en_m2:
                    continue
                bcols = slice((pair * 2 + bi) * K * P, (pair * 2 + bi + 1) * K * P)
                mj0 = mjp.tile([128, K * P], F32, tag="mj0")
                nc.vector.scalar_tensor_tensor(
                    out=mj0, in0=ps[(bi, 0)][:, :], scalar=1.0,
                    in1=gyT1[:, bcols], op0=ALU.mult, op1=ALU.mult,
                    accum_out=stats_dve[:, _c_m2(pair, bi, 0):_c_m2(pair, bi, 0) + 1])
                mj1 = mjp.tile([64, K * P], F32, tag="mj1")
                nc.vector.scalar_tensor_tensor(
                    out=mj1, in0=ps[(bi, 1)][:, :], scalar=1.0,
                    in1=gyT2[0:64, bcols], op0=ALU.mult, op1=ALU.mult,
                    accum_out=stats_dve[0:64, _c_m2(pair, bi, 1):_c_m2(pair, bi, 1) + 1])


        if en_bce and not bce_first:
            emit_bce()

        # ---- write partials ----
        nc.sync.dma_start(out=out_d[:, 0:64], in_=stats_act[:, :])
        nc.sync.dma_start(out=out_d[:, 64:128], in_=stats_dve[:, :])
        nc.sync.dma_start(out=out_d[:, 128:160], in_=stats_gp[:, :])

    nc.compile()
    return nc


_NC_CACHE = {}


def _get_nc(s2_engine="act"):
    if s2_engine not in _NC_CACHE:
        _NC_CACHE[s2_engine] = build_nc(s2_engine)
    return _NC_CACHE[s2_engine]


def host_prep_core(keypoints, visibilities):
    """Per-core host preprocessing from the tiny keypoint tensors.

    Returns (bx[128,KP_TILES], by[128,KP_TILES], T2[BPC] float64, denom[BPC]).
    Matches reference semantics exactly: x = floor(f32(kx * 191)),
    valid = (vis > 0) & (0 <= x < W) & (0 <= y < H). gx carries the valid
    mask (via bias = 1e9 so exp underflows to exactly 0), gy does not.
    """
    kx = keypoints[..., 0].astype(np.float32) * np.float32(W - 1)
    ky = keypoints[..., 1].astype(np.float32) * np.float32(H - 1)
    x = np.floor(kx)
    y = np.floor(ky)
    valid = ((visibilities > 0) & (x >= 0) & (x < W) & (y >= 0) & (y < H))

    # bias rows ordered (b, k, p) to match the gaussian column order
    bx = np.full(KP_TILES * 128, 1e9, dtype=np.float32)
    by = np.full(KP_TILES * 128, 1e9, dtype=np.float32)
    xr = np.transpose(x, (0, 2, 1)).reshape(-1)       # [b,k,p] flat
    yr = np.transpose(y, (0, 2, 1)).reshape(-1)
    vr = np.transpose(valid, (0, 2, 1)).reshape(-1)
    bx[:KP] = np.where(vr, -xr, np.float32(1e9))
    by[:KP] = -yr  # gy has no valid mask in the reference
    bx = bx.reshape(KP_TILES, 128).T.copy()           # [128, KP_TILES]
    by = by.reshape(KP_TILES, 128).T.copy()

    # T2 = sum over target^2, in float64 on host (keypoints-only quantity)
    ax = np.arange(W, dtype=np.float64)
    gx = np.exp(-((ax[None, None, None, :] - x[..., None].astype(np.float64)) ** 2)
                * INV2S2) * valid[..., None]          # [BPC,P,K,W]
    gy = np.exp(-((ax[None, None, None, :] - y[..., None].astype(np.float64)) ** 2)
                * INV2S2)                             # [BPC,P,K,H]
    gxg = np.einsum("bpki,bqki->bkpq", gx, gx)
    gyg = np.einsum("bpkj,bqkj->bkpq", gy, gy)
    T2 = np.einsum("bkpq,bkpq->b", gxg, gyg)

    denom = visibilities.sum(axis=(1, 2)).astype(np.float64) + 1e-6
    return bx, by, T2, denom


def core_sample_stats(pa, sloc):
    """Extract per-sample scalar stats from one core's [128, cols] partials."""
    pa = pa.astype(np.float64)
    act, dve, gp = pa[:, 0:64], pa[:, 64:128], pa[:, 128:160]
    pair, bi = sloc // 2, sloc % 2
    Zs = Zt = A1 = A2 = S2 = 0.0
    for ci in range(3):
        sA = _slot(pair, ci, bi)     # own h<128 group: all partitions
        sB = _slot(pair, ci, 2)      # packed h>=128 group: own half
        hp = slice(64 * bi, 64 * bi + 64)
        Zs += act[:, _c_zs(sA)].sum() + act[hp, _c_zs(sB)].sum()
        Zt += act[:, _c_zt(sA)].sum() + act[hp, _c_zt(sB)].sum()
        A1 += dve[:, _c_a1(sA)].sum() + dve[hp, _c_a1(sB)].sum()
        A2 += dve[:, _c_a2(sA)].sum() + dve[hp, _c_a2(sB)].sum()
        S2 += gp[:, _c_s2(sA)].sum() + gp[hp, _c_s2(sB)].sum()
    M2 = (dve[:, _c_m2(pair, bi, 0)].sum()
          + dve[0:64, _c_m2(pair, bi, 1)].sum())
    return dict(Zs=Zs, Zt=Zt, A1=A1, A2=A2, S2=S2, M2=M2)


def host_reduce(partials_list, T2_list, denom_list):
    """Combine per-core [128, PARTIALS_COLS] partials into the final loss."""
    kl_sum = 0.0
    sp_sum = 0.0
    xm_sum = 0.0
    pose_terms = []
    for c in range(NCORES):
        pa = partials_list[c].astype(np.float64)
        sp_sum += pa[:, C_SP].sum()
        xm_sum += pa[:, 64 + C_XM].sum()
        for sloc in range(BPC):
            st = core_sample_stats(partials_list[c], sloc)
            kl_sum += (st["A1"] / (TEMP * st["Zt"])
                       - np.log(st["Zt"]) + np.log(st["Zs"]))
            sse = st["S2"] - 2.0 * st["M2"] + T2_list[c][sloc]
            pose_terms.append(sse / denom_list[c][sloc])

    pose_distill = (TEMP ** 2) * kl_sum / B
    task_seg = (sp_sum - xm_sum) / (B * H * W)
    task_pose = float(np.mean(pose_terms))
    total = ALPHA * pose_distill + (1.0 - ALPHA) * (task_seg + task_pose)
    return np.float32(total)


def make_in_maps(s_seg_logits, s_pose_logits, t_pose_logits, mask,
                 keypoints, visibilities):
    in_maps, T2s, denoms = [], [], []
    for c in range(NCORES):
        sl = slice(BPC * c, BPC * (c + 1))
        bx, by, T2, denom = host_prep_core(keypoints[sl], visibilities[sl])
        in_maps.append({
            "s_pose": np.ascontiguousarray(s_pose_logits[sl]),
            "t_pose": np.ascontiguousarray(t_pose_logits[sl]),
            "s_seg": np.ascontiguousarray(s_seg_logits[sl, 0]),
            "mask": np.ascontiguousarray(mask[sl]),
            "bx": bx,
            "by": by,
        })
        T2s.append(T2)
        denoms.append(denom)
    return in_maps, T2s, denoms


def kernel(s_seg_logits, s_pose_logits, t_seg_logits, t_pose_logits,
           mask, keypoints, visibilities):
    nc = _get_nc()
    in_maps, T2s, denoms = make_in_maps(
        s_seg_logits, s_pose_logits, t_pose_logits, mask,
        keypoints, visibilities)
    res = run_bass_kernel_spmd(nc, in_maps, core_ids=list(range(NCORES)))
    partials = [r["partials"] for r in res.results]
    return host_reduce(partials, T2s, denoms)
